# revision 14
# baseline (speedup 1.0000x reference)
"""Trainium2 Bass kernel for a 4-layer sliding-window-attention transformer.

Sharding: 8-way data parallel over the sequence (8192 tokens -> 1024 owned
tokens per core) with redundant halo recompute (1024-token halo at the left
edge shrinking by 256 tokens per layer), so no collectives are needed.

On-chip layout: activations flow TRANSPOSED ([dim, token]) so every linear
runs as lhsT(=weight)[K,M].T @ rhs(=activation)[K,N] on the PE.  All matmul
operands are float32r (full-rate on PE, ~1e-4 rounding).  RoPE head dims are
host-permuted into even/odd groups of 16 inside each 32-partition quadrant so
the rotation partner is a DVE stream_shuffle away.  Softmax runs without max
subtraction (scores are bounded for this model family); masking is a
multiplicative 0/1 mask on exp(scores); the denominator comes free from an
all-ones 65th column appended to V.
"""

import sys

sys.path.insert(0, "/opt/trn_rl_repo")

import numpy as np

DIM = 1024
HEAD_DIM = 64
N_HEADS = 16
N_KV_HEADS = 4
HIDDEN = 4096
N_LAYERS = 4
WINDOW = 256
INPUT_DIM = 256
NUM_CLASSES = 2000
ROPE_THETA = 10000.0
EPS = 1e-6

S = 8192
NCORE = 8
OWN = S // NCORE          # 1024 owned tokens per core
T = 2 * OWN               # 2048 local tokens (1024 halo + 1024 owned)
NT = T // 128             # 16 token tiles
CH = 512                  # working chunk (tokens)

SWAP_MASK = list(range(16, 32)) + list(range(0, 16))

_COMPILED = None


def _chunks(start, end, step=CH):
    out = []
    s = start
    while s < end:
        out.append((s, min(step, end - s)))
        s += step
    return out


# --------------------------------------------------------------------------
# device program
# --------------------------------------------------------------------------

def _build(debug=False, stages=99):
    import concourse.bacc as bacc
    import concourse.mybir as mybir
    from concourse.tile import TileContext

    F32 = mybir.dt.float32
    F32R = mybir.dt.float32r
    AF = mybir.ActivationFunctionType
    OP = mybir.AluOpType

    nc = bacc.Bacc("TRN2", target_bir_lowering=False, debug=False,
                   num_devices=NCORE)

    # ---- DRAM parameters --------------------------------------------------
    xT_d = nc.dram_tensor("xT", [128, 2, T], F32R, kind="ExternalInput")
    cos_d = nc.dram_tensor("cosB", [128, T], F32R, kind="ExternalInput")
    sin_d = nc.dram_tensor("sinB", [128, T], F32R, kind="ExternalInput")
    mask_d = nc.dram_tensor("masks", [NT, 128, 384], F32R, kind="ExternalInput")
    embw_d = nc.dram_tensor("emb_w", [128, 2, DIM], F32R, kind="ExternalInput")
    embb_d = nc.dram_tensor("emb_b", [128, 8], F32, kind="ExternalInput")
    wqkv_d = nc.dram_tensor("wqkv", [N_LAYERS, 128, 8, 1536], F32R, kind="ExternalInput")
    wo_d = nc.dram_tensor("wo", [N_LAYERS, 128, 8, DIM], F32R, kind="ExternalInput")
    w1_d = nc.dram_tensor("w1", [N_LAYERS, 128, 8, HIDDEN], F32R, kind="ExternalInput")
    w3_d = nc.dram_tensor("w3", [N_LAYERS, 128, 8, HIDDEN], F32R, kind="ExternalInput")
    w2_d = nc.dram_tensor("w2", [N_LAYERS, 32, 128, DIM], F32R, kind="ExternalInput")
    ones_d = nc.dram_tensor("ones_row", [1, 64], F32R, kind="ExternalInput")
    oneb_d = nc.dram_tensor("ones_blk", [128, 128], F32R, kind="ExternalInput")
    idn_d = nc.dram_tensor("idn", [128, 128], F32R, kind="ExternalInput")
    ones4_d = nc.dram_tensor("ones4", [128, 4], F32R, kind="ExternalInput")
    pool_d = nc.dram_tensor("pooled", [8, 128], F32, kind="ExternalOutput")
    if debug:
        dbg_d = nc.dram_tensor("dbg_h", [N_LAYERS + 1, 128, 8, T], F32,
                               kind="ExternalOutput")

    with TileContext(nc) as tc:
        with tc.tile_pool(name="persist", bufs=1) as pp:
            h_all = pp.tile([128, 8, T], F32R, tag="h")
            k_all = pp.tile([128, 2, T], F32R, tag="k")
            v_aug = pp.tile([128, NT, 260], F32R, tag="v")
            sbc = pp.tile([128, T], F32R, tag="sbc")
            ones_r = pp.tile([1, 64], F32R, tag="ones")
            ones_b = pp.tile([128, 128], F32R, tag="oneb")
            idn = pp.tile([128, 128], F32R, tag="idn")
            ebias = pp.tile([128, 8], F32, tag="ebias")
            nc.sync.dma_start(ones_r[:], ones_d.ap())
            nc.sync.dma_start(ones_b[:], oneb_d.ap())
            nc.sync.dma_start(idn[:], idn_d.ap())
            nc.sync.dma_start(ebias[:], embb_d.ap())

            def rms_stats(sp, start_tok, ppool, x2pool, nrmpool):
                # sbc[:, start_tok:T] = rsqrt(mean(h^2) + eps), replicated rows
                for (cs, cl) in _chunks(start_tok, T):
                    ps = ppool.tile([128, cl], F32, tag="pmm")
                    for dt in range(8):
                        x2 = x2pool.tile([128, cl], F32R, tag="x2")
                        nc.vector.tensor_tensor(
                            x2[:], h_all[:, dt, cs:cs + cl],
                            h_all[:, dt, cs:cs + cl], OP.mult)
                        nc.tensor.matmul(ps[:], ones_b[:], x2[:],
                                         start=(dt == 0), stop=(dt == 7))
                    t1 = nrmpool.tile([128, cl], F32, tag="nrm1", bufs=1)
                    nc.vector.tensor_scalar(t1[:], ps[:], 1.0 / DIM, EPS,
                                            OP.mult, OP.add)
                    t2 = nrmpool.tile([128, cl], F32, tag="nrm2", bufs=1)
                    nc.vector.reciprocal(t2[:], t1[:])
                    nc.scalar.activation(sbc[:, cs:cs + cl], t2[:], AF.Sqrt)

            # ---- embedding ----------------------------------------------
            with tc.tile_pool(name="emb", bufs=1) as pe, \
                 tc.tile_pool(name="embp", bufs=2, space="PSUM") as ppe:
                ew = pe.tile([128, 2, DIM], F32R, tag="embw")
                nc.sync.dma_start(ew[:], embw_d.ap())
                for (cs, cl) in _chunks(0, T):
                    xw = pe.tile([128, 2, cl], F32R, tag="xT", bufs=2)
                    nc.sync.dma_start(xw[:], xT_d.ap()[:, :, cs:cs + cl])
                    for mt in range(8):
                        ps = ppe.tile([128, cl], F32, tag="pmm")
                        for kt in range(2):
                            nc.tensor.matmul(
                                ps[:], ew[:, kt, mt * 128:(mt + 1) * 128],
                                xw[:, kt, :],
                                start=(kt == 0), stop=(kt == 1))
                        nc.scalar.activation(h_all[:, mt, cs:cs + cl], ps[:],
                                             AF.Identity,
                                             bias=ebias[:, mt:mt + 1])
            if debug:
                for dt in range(8):
                    nc.sync.dma_start(dbg_d.ap()[0, :, dt, :],
                                      h_all[:, dt, :].bitcast(F32))

            # ---- layers -------------------------------------------------
            for l in range(N_LAYERS):
                if 2 * l + 1 > stages:
                    break
                Kl = 256 * l
                Ll = 256 * (l + 1)

                # ======== phase B: rms1 + qkv + rope + attention + wo ====
                with tc.tile_pool(name=f"b1_{l}", bufs=1) as pb1, \
                     tc.tile_pool(name=f"b2_{l}", bufs=2) as pb2, \
                     tc.tile_pool(name=f"b3_{l}", bufs=3) as pb3, \
                     tc.tile_pool(name=f"bp_{l}", bufs=2, space="PSUM") as ppB, \
                     tc.tile_pool(name=f"bs_{l}", bufs=2, space="PSUM") as ppS:

                    rms_stats(sbc, Kl, ppB, pb2, pb2)

                    for (cs, cl) in _chunks(Kl, T):
                        # ---- qkv projections (12 m-tiles: 8 q, 2 k, 2 v)
                        q_ch = pb2.tile([128, 8, cl], F32R, tag="qch", bufs=1)
                        v_T = pb2.tile([128, 2, cl], F32R, tag="vT")
                        for g in range(6):
                            wt = pb2.tile([128, 8, 256], F32R, tag="wst")
                            nc.sync.dma_start(
                                wt[:], wqkv_d.ap()[l, :, :, g * 256:(g + 1) * 256])
                            for m2 in range(2):
                                mt = g * 2 + m2
                                ps = ppB.tile([128, cl], F32, tag="pmm")
                                for kt in range(8):
                                    nc.tensor.matmul(
                                        ps[:], wt[:, kt, m2 * 128:(m2 + 1) * 128],
                                        h_all[:, kt, cs:cs + cl],
                                        start=(kt == 0), stop=(kt == 7))
                                if mt < 8:
                                    dst = q_ch[:, mt, :]
                                elif mt < 10:
                                    dst = k_all[:, mt - 8, cs:cs + cl]
                                else:
                                    dst = v_T[:, mt - 10, :]
                                nc.vector.tensor_tensor(
                                    dst, ps[:], sbc[:, cs:cs + cl], OP.mult)

                        # ---- rope on q and k
                        cos_t = pb2.tile([128, cl], F32R, tag="cos")
                        sin_t = pb2.tile([128, cl], F32R, tag="sin")
                        nc.sync.dma_start(cos_t[:], cos_d.ap()[:, cs:cs + cl])
                        nc.sync.dma_start(sin_t[:], sin_d.ap()[:, cs:cs + cl])

                        def rope(xap):
                            sh = pb2.tile([128, cl], F32, tag="shuf")
                            nc.vector.stream_shuffle(sh[:], xap, SWAP_MASK)
                            nc.vector.tensor_tensor(xap, xap, cos_t[:], OP.mult)
                            nc.vector.tensor_tensor(sh[:], sh[:], sin_t[:], OP.mult)
                            nc.vector.tensor_tensor(xap, xap, sh[:], OP.add)

                        for dt in range(8):
                            rope(q_ch[:, dt, :])
                        for dt in range(2):
                            rope(k_all[:, dt, cs:cs + cl])

                        # ---- v transpose into v_aug (+ ones column)
                        for tti in range(cl // 128):
                            tt = cs // 128 + tti
                            for dvt in range(2):
                                pt = ppS.tile([128, 128], F32R, tag="ptr", bufs=1)
                                nc.tensor.transpose(
                                    pt[:], v_T[:, dvt, tti * 128:(tti + 1) * 128],
                                    idn[:])
                                dst = v_aug[:, tt, 2 * dvt * 65:2 * dvt * 65 + 130]
                                dst = dst.rearrange("p (h c) -> p h c", c=65)[:, :, 0:64]
                                nc.vector.tensor_copy(
                                    dst, pt[:].rearrange("p (h c) -> p h c", c=64))
                            oc = v_aug[:, tt, :].rearrange(
                                "p (h c) -> p h c", c=65)[:, :, 64:65]
                            nc.sync.dma_start(oc, ones4_d.ap().unsqueeze(2))

                        # ---- attention for q-tiles in [max(cs, Ll), cs+cl)
                        aq = max(cs, Ll)
                        aql = cs + cl - aq
                        if aql <= 0:
                            continue
                        attn_ch = pb1.tile([128, 8, aql], F32R, tag="attn")
                        for t in range(aq // 128, (cs + cl) // 128):
                            mk = pb2.tile([128, 384], F32R, tag="mask")
                            nc.sync.dma_start(mk[:], mask_d.ap()[t])
                            qo = t * 128 - cs
                            for h in range(N_HEADS):
                                dtq = h % 4 + 4 * (h // 8)
                                b = 64 * ((h // 4) % 2)
                                kv = h // 4
                                dtk = kv // 2
                                pS = ppS.tile([128, 384], F32, tag="pscore")
                                for j in range(3):
                                    kt_t = t - 2 + j
                                    nc.tensor.matmul(
                                        pS[:, j * 128:(j + 1) * 128],
                                        k_all[b:b + 64, dtk,
                                              kt_t * 128:(kt_t + 1) * 128],
                                        q_ch[b:b + 64, dtq, qo:qo + 128],
                                        start=True, stop=True)
                                ex = pb3.tile([128, 384], F32R, tag="exps")
                                nc.scalar.activation(ex[:], pS[:], AF.Exp)
                                nc.vector.tensor_tensor(ex[:], ex[:], mk[:],
                                                        OP.mult)
                                pO = ppS.tile([65, 128], F32, tag="ppv")
                                for j in range(3):
                                    nc.tensor.matmul(
                                        pO[:],
                                        v_aug[:, t - 2 + j, kv * 65:kv * 65 + 65],
                                        ex[:, j * 128:(j + 1) * 128],
                                        start=(j == 0), stop=(j == 2))
                                rc = pb3.tile([1, 128], F32R, tag="rcp")
                                with nc.allow_low_precision(reason="f32r recip feeds PE bcast"):
                                    nc.vector.reciprocal(rc[:], pO[64:65, :])
                                pB = ppS.tile([64, 128], F32, tag="pbc", bufs=1)
                                nc.tensor.matmul(pB[:], ones_r[:], rc[:],
                                                 start=True, stop=True)
                                rb = pb3.tile([64, 128], F32, tag="rbc",
                                              bufs=2)
                                nc.scalar.copy(rb[:], pB[:])
                                nc.vector.tensor_tensor(
                                    attn_ch[b:b + 64, dtq,
                                            t * 128 - aq:t * 128 - aq + 128],
                                    pO[0:64, :], rb[:], OP.mult)

                        # ---- wo + residual
                        for g in range(4):
                            wt = pb2.tile([128, 8, 256], F32R, tag="wst")
                            nc.sync.dma_start(
                                wt[:], wo_d.ap()[l, :, :, g * 256:(g + 1) * 256])
                            for m2 in range(2):
                                mt = g * 2 + m2
                                ps = ppB.tile([128, aql], F32, tag="pmm")
                                for kt in range(8):
                                    nc.tensor.matmul(
                                        ps[:], wt[:, kt, m2 * 128:(m2 + 1) * 128],
                                        attn_ch[:, kt, :],
                                        start=(kt == 0), stop=(kt == 7))
                                nc.vector.tensor_tensor(
                                    h_all[:, mt, aq:aq + aql],
                                    h_all[:, mt, aq:aq + aql], ps[:], OP.add)

                if debug and 2 * l + 2 > stages:
                    for dt in range(8):
                        nc.sync.dma_start(dbg_d.ap()[l + 1, :, dt, :],
                                          h_all[:, dt, :].bitcast(F32))
                # ======== phase D: rms2 + ffn ============================
                if 2 * l + 2 > stages:
                    break
                with tc.tile_pool(name=f"d1_{l}", bufs=1) as pd1, \
                     tc.tile_pool(name=f"d2_{l}", bufs=2) as pd2, \
                     tc.tile_pool(name=f"dp_{l}", bufs=2, space="PSUM") as ppD, \
                     tc.tile_pool(name=f"dh_{l}", bufs=4, space="PSUM") as ppH:

                    rms_stats(sbc, Ll, ppD, pd2, pd2)

                    for (cs, cl) in _chunks(Ll, T):
                        g_buf = pd1.tile([128, 32, cl], F32R, tag="gbuf")
                        for m in range(32):
                            wt1 = pd2.tile([128, 8, 128], F32R, tag="w13a")
                            wt3 = pd2.tile([128, 8, 128], F32R, tag="w13b")
                            nc.sync.dma_start(
                                wt1[:], w1_d.ap()[l, :, :, m * 128:(m + 1) * 128])
                            nc.sync.dma_start(
                                wt3[:], w3_d.ap()[l, :, :, m * 128:(m + 1) * 128])
                            psu = ppD.tile([128, cl], F32, tag="pmm")
                            for kt in range(8):
                                nc.tensor.matmul(psu[:], wt1[:, kt, :],
                                                 h_all[:, kt, cs:cs + cl],
                                                 start=(kt == 0), stop=(kt == 7))
                            us = pd2.tile([128, cl], F32R, tag="usc", bufs=1)
                            nc.vector.tensor_tensor(us[:], psu[:],
                                                    sbc[:, cs:cs + cl], OP.mult)
                            nc.scalar.activation(g_buf[:, m, :], us[:], AF.Silu)
                            psv = ppD.tile([128, cl], F32, tag="pmm")
                            for kt in range(8):
                                nc.tensor.matmul(psv[:], wt3[:, kt, :],
                                                 h_all[:, kt, cs:cs + cl],
                                                 start=(kt == 0), stop=(kt == 7))
                            nc.vector.tensor_tensor(g_buf[:, m, :],
                                                    g_buf[:, m, :], psv[:],
                                                    OP.mult)

                        # w2: h2_normal[t, d] = sum_m g[m, t] * w2[m, d]
                        nts = cl // 128
                        sc_tiles = {}
                        for half in range(2):
                            psh = [ppH.tile([128, 512], F32, tag="ph2",
                                            name=f"ph2_{l}_{cs}_{half}_{i}")
                                   for i in range(nts)]
                            for m in range(32):
                                w2t = pd2.tile([128, 512], F32R, tag="w2t")
                                nc.sync.dma_start(
                                    w2t[:],
                                    w2_d.ap()[l, m, :, half * 512:(half + 1) * 512])
                                for ts in range(nts):
                                    nc.tensor.matmul(
                                        psh[ts][:],
                                        g_buf[:, m, ts * 128:(ts + 1) * 128],
                                        w2t[:], start=(m == 0), stop=(m == 31))
                            for ts in range(nts):
                                # s2 column for this token sub-tile
                                if half == 0:
                                    ptc = ppD.tile([128, 128], F32R, tag="ptr2")
                                    nc.tensor.transpose(
                                        ptc[:],
                                        sbc[:, cs + ts * 128: cs + (ts + 1) * 128],
                                        idn[:])
                                    sc = pd2.tile([128, 1], F32, tag=f"sc{ts}")
                                    nc.vector.tensor_copy(sc[:], ptc[:, 0:1])
                                    sc_tiles[ts] = sc
                                else:
                                    sc = sc_tiles[ts]
                                h2s = pd2.tile([128, 512], F32R, tag="h2sb")
                                nc.vector.tensor_scalar(
                                    h2s[:], psh[ts][:], sc[:], None, OP.mult)
                                for db in range(4):
                                    pt2 = ppD.tile([128, 128], F32R, tag="ptr2")
                                    nc.tensor.transpose(
                                        pt2[:], h2s[:, db * 128:(db + 1) * 128],
                                        idn[:])
                                    dg = half * 4 + db
                                    dst = h_all[:, dg,
                                                cs + ts * 128: cs + (ts + 1) * 128]
                                    nc.vector.tensor_tensor(dst, dst, pt2[:],
                                                            OP.add)


                if debug:
                    for dt in range(8):
                        nc.sync.dma_start(dbg_d.ap()[l + 1, :, dt, :],
                                          h_all[:, dt, :].bitcast(F32))

            # ---- final norm + pooling -----------------------------------
            with tc.tile_pool(name="fin", bufs=2) as pf, \
                 tc.tile_pool(name="finp", bufs=2, space="PSUM") as ppF:
                rms_stats(sbc, OWN, ppF, pf, pf)
                for dt in range(8):
                    scr = pf.tile([128, OWN], F32, tag="fscr")
                    nc.vector.tensor_tensor(
                        scr[:], h_all[:, dt, OWN:T].bitcast(F32),
                        sbc[:, OWN:T].bitcast(F32), OP.mult)
                    a2 = pf.tile([128, 1], F32, tag="facc2")
                    nc.vector.reduce_sum(a2[:], scr[:],
                                         axis=mybir.AxisListType.X)
                    nc.sync.dma_start(pool_d.ap()[dt:dt + 1, :], a2[:])

    nc.compile()
    return nc


# --------------------------------------------------------------------------
# host-side input preparation
# --------------------------------------------------------------------------

def _new_row_to_orig(r):
    # within-head new row r (0..63) -> original head dim
    qd, rr = divmod(r, 32)
    p = qd * 16 + (rr % 16)
    return 2 * p + (1 if rr >= 16 else 0)


_ROW2ORIG = np.array([_new_row_to_orig(r) for r in range(HEAD_DIM)])


def _q_col_perm():
    # new q column -> original wq column
    perm = np.zeros(DIM, dtype=np.int64)
    for h in range(N_HEADS):
        dt = h % 4 + 4 * (h // 8)
        base = 64 * ((h // 4) % 2)
        for r in range(HEAD_DIM):
            perm[dt * 128 + base + r] = h * HEAD_DIM + _ROW2ORIG[r]
    return perm


def _k_col_perm():
    perm = np.zeros(N_KV_HEADS * HEAD_DIM, dtype=np.int64)
    for kv in range(N_KV_HEADS):
        dt = kv // 2
        base = 64 * (kv % 2)
        for r in range(HEAD_DIM):
            perm[dt * 128 + base + r] = kv * HEAD_DIM + _ROW2ORIG[r]
    return perm


def _v_col_perm():
    perm = np.zeros(N_KV_HEADS * HEAD_DIM, dtype=np.int64)
    for kv in range(N_KV_HEADS):
        dt = kv // 2
        base = 64 * (kv % 2)
        for r in range(HEAD_DIM):
            perm[dt * 128 + base + r] = kv * HEAD_DIM + r
    return perm


def _wo_row_perm():
    # new attn row -> original wo row
    perm = np.zeros(DIM, dtype=np.int64)
    for h in range(N_HEADS):
        dt = h % 4 + 4 * (h // 8)
        base = 64 * ((h // 4) % 2)
        for r in range(HEAD_DIM):
            perm[dt * 128 + base + r] = h * HEAD_DIM + r
    return perm


def _kxm_pack(w):
    # [K, M] -> [128, K//128, M]
    K, M = w.shape
    return np.ascontiguousarray(
        w.reshape(K // 128, 128, M).transpose(1, 0, 2))


def _rope_tables(gbase):
    half = HEAD_DIM // 2
    inv_freq = 1.0 / (ROPE_THETA ** (np.arange(half, dtype=np.float64) / half))
    g = np.maximum(gbase + np.arange(T), 0).astype(np.float64)
    ang = g[None, :] * inv_freq[:, None]        # [32, T]
    cos = np.cos(ang)
    sin = np.sin(ang)
    cosB = np.zeros((128, T), np.float32)
    sinB = np.zeros((128, T), np.float32)
    for r in range(128):
        rh = r % HEAD_DIM
        p = (rh // 32) * 16 + (rh % 32) % 16
        sign = -1.0 if (rh % 32) < 16 else 1.0
        cosB[r] = cos[p]
        sinB[r] = sign * sin[p]
    return cosB, sinB


def _masks(gbase):
    m = np.zeros((NT, 128, 384), np.float32)
    qi = np.arange(128)
    kp = np.arange(128)
    for t in range(NT):
        gq = gbase + t * 128 + qi            # [128]
        for j in range(3):
            gk = gbase + (t - 2 + j) * 128 + kp   # [128]
            valid = ((gk[:, None] >= 0) & (gk[:, None] <= gq[None, :])
                     & (gk[:, None] >= gq[None, :] - (WINDOW - 1)))
            # halo region of core 0 (gq < 0): generic pattern, keeps denom > 0
            if j == 0:
                gen = kp[:, None] > qi[None, :]
            elif j == 1:
                gen = np.ones((128, 128), bool)
            else:
                gen = kp[:, None] <= qi[None, :]
            use = np.where(gq[None, :] >= 0, valid, gen)
            m[t, :, j * 128:(j + 1) * 128] = use.astype(np.float32)
    return m


def _prep_inputs(x, params):
    x = np.asarray(x, np.float32)
    P = {k: np.asarray(v, np.float32) if not isinstance(v, (list, int)) else v
         for k, v in params.items()}
    layers = params["layers"]

    qp = _q_col_perm()
    kp_ = _k_col_perm()
    vp = _v_col_perm()
    wop = _wo_row_perm()

    wqkv = np.zeros((N_LAYERS, 128, 8, 1536), np.float32)
    wo = np.zeros((N_LAYERS, 128, 8, DIM), np.float32)
    w1 = np.zeros((N_LAYERS, 128, 8, HIDDEN), np.float32)
    w3 = np.zeros((N_LAYERS, 128, 8, HIDDEN), np.float32)
    w2 = np.zeros((N_LAYERS, 32, 128, DIM), np.float32)
    for l, lp in enumerate(layers):
        anorm = np.asarray(lp["attn_norm"], np.float32)
        fnorm = np.asarray(lp["ffn_norm"], np.float32)
        wq_p = (anorm[:, None] * np.asarray(lp["wq"], np.float32))[:, qp] * (HEAD_DIM ** -0.5)
        wk_p = (anorm[:, None] * np.asarray(lp["wk"], np.float32))[:, kp_]
        wv_p = (anorm[:, None] * np.asarray(lp["wv"], np.float32))[:, vp]
        wqkv[l] = _kxm_pack(np.concatenate([wq_p, wk_p, wv_p], axis=1))
        wo[l] = _kxm_pack(np.asarray(lp["wo"], np.float32)[wop, :])
        w1[l] = _kxm_pack(fnorm[:, None] * np.asarray(lp["w1"], np.float32))
        w3[l] = _kxm_pack(fnorm[:, None] * np.asarray(lp["w3"], np.float32))
        w2l = np.asarray(lp["w2"], np.float32)           # [4096, 1024]
        w2[l] = w2l.reshape(32, 128, DIM)

    emb_w = _kxm_pack(np.asarray(P["emb_w"], np.float32))   # [128, 2, 1024]
    emb_b = np.asarray(P["emb_b"], np.float32).reshape(8, 128).T.copy()  # [128, 8]

    common = dict(
        wqkv=wqkv, wo=wo, w1=w1, w3=w3, w2=w2,
        emb_w=emb_w, emb_b=emb_b,
        ones_row=np.ones((1, 64), np.float32),
        ones_blk=np.ones((128, 128), np.float32),
        idn=np.eye(128, dtype=np.float32),
        ones4=np.ones((128, 4), np.float32),
    )

    xs = x[0]                                   # [S, INPUT_DIM]
    in_maps = []
    for c in range(NCORE):
        gbase = c * OWN - OWN
        xT = np.zeros((INPUT_DIM, T), np.float32)
        lo = max(0, gbase)
        xT[:, lo - gbase:T] = xs[lo:gbase + T].T
        xTp = np.ascontiguousarray(
            xT.reshape(2, 128, T).transpose(1, 0, 2))    # [128, 2, T]
        cosB, sinB = _rope_tables(gbase)
        im = dict(common)
        im["xT"] = xTp
        im["cosB"] = cosB
        im["sinB"] = sinB
        im["masks"] = _masks(gbase)
        in_maps.append(im)
    return in_maps


def _host_head(results, params):
    norm = np.asarray(params["norm"], np.float32)
    out_w = np.asarray(params["out_w"], np.float32)
    out_b = np.asarray(params["out_b"], np.float32)
    pooled = np.zeros(DIM, np.float64)
    for r in results:
        pooled += r["pooled"].reshape(DIM).astype(np.float64)
    pooled = (pooled / S).astype(np.float32)
    logits = (pooled * norm) @ out_w + out_b
    return logits[None, :].astype(np.float32)


def kernel(x=None, seqlens=None, params=None, **kw):
    global _COMPILED
    from concourse.bass_utils import run_bass_kernel_spmd
    if _COMPILED is None:
        _COMPILED = _build()
    in_maps = _prep_inputs(x, params)
    res = run_bass_kernel_spmd(_COMPILED, in_maps, list(range(NCORE)))
    return _host_head(res.results, params)
